# revision 1
# baseline (speedup 1.0000x reference)
"""GCN 3-layer message-passing kernel for TRN2 (8 NeuronCores, SPMD).

Self-contained: takes FULL inputs, shards internally, runs a Bass/Tile
kernel via run_bass_kernel_spmd on cores 0-7, gathers the full output.

Strategy:
  - Target nodes partitioned into 8 contiguous shards (random graph, so
    contiguous == random partition; no METIS needed).
  - Per layer: local transform (h_shard @ W) -> AllGather transformed
    table -> per-target-block dma_gather of source rows (int16-chunked)
    -> one-hot matmul segment-sum in PSUM -> bias (identity matmul) +
    relu fused into the ACT PSUM evacuation.
  - Edge metadata (chunk-local indices, within-block target slot, GCN
    norm) precomputed on host; a shared slot schedule keeps the SPMD
    program identical across cores while edge data differs per core.
"""

import os
import sys

sys.path.insert(0, "/opt/trn_rl_repo")

import numpy as np

import concourse.bass as bass  # noqa: F401
import concourse.mybir as mybir
import concourse.tile as tile
from concourse import bacc
from concourse._compat import cdiv
from concourse.bass_utils import run_bass_kernel_spmd

F32 = mybir.dt.float32
I16 = mybir.dt.int16
AL = mybir.AluOpType
AF = mybir.ActivationFunctionType

NC = 8
P = 128
CHUNK = 25600  # int16-addressable rows per gather table chunk

LAST_EXEC_NS = None


def _cdiv_np(a, b):
    return (a + b - 1) // b


def _group_cumcount(grp: np.ndarray) -> np.ndarray:
    n = len(grp)
    if n == 0:
        return np.zeros(0, dtype=np.int64)
    is_new = np.ones(n, dtype=bool)
    is_new[1:] = grp[1:] != grp[:-1]
    idx = np.arange(n)
    start = np.maximum.accumulate(np.where(is_new, idx, 0))
    return idx - start


def _preprocess(edge_index: np.ndarray, n_nodes: int, chunk: int):
    N = n_nodes
    S = N // NC
    n_blocks = cdiv(S, P)
    nq = cdiv(N, chunk)

    src = np.concatenate([edge_index[0], np.arange(N, dtype=np.int64)])
    dst = np.concatenate([edge_index[1], np.arange(N, dtype=np.int64)])
    deg = np.bincount(dst, minlength=N).astype(np.float64)
    dis = 1.0 / np.sqrt(deg)
    norm = (dis[src] * dis[dst]).astype(np.float32)

    core = dst // S
    block = (dst % S) // P
    t_local = (dst % S) % P
    q = src // chunk
    src_local = src - q * chunk

    counts = np.zeros((NC, n_blocks, nq), dtype=np.int64)
    np.add.at(counts, (core, block, q), 1)
    slots = np.maximum(1, _cdiv_np(counts.max(axis=0), P))

    k_b = slots.sum(axis=1)
    K_total = int(k_b.sum())
    IW = K_total * 8

    tn_off = np.zeros(n_blocks, dtype=np.int64)
    tn_off[1:] = np.cumsum(2 * k_b)[:-1]
    ix_off = np.zeros((n_blocks, nq), dtype=np.int64)
    gslot_off = np.zeros((n_blocks, nq), dtype=np.int64)
    acc = 0
    for b in range(n_blocks):
        sacc = 0
        for qq in range(nq):
            ix_off[b, qq] = acc
            gslot_off[b, qq] = sacc
            acc += int(slots[b, qq]) * 8
            sacc += int(slots[b, qq])
    assert acc == IW

    order = np.lexsort((src, q, block, core))
    so_src_local = src_local[order]
    so_norm = norm[order]
    so_tl = t_local[order]
    so_core = core[order]
    so_block = block[order]
    so_q = q[order]

    per_core = []
    for c in range(NC):
        m = so_core == c
        cb, cq = so_block[m], so_q[m]
        csl, cn, ctl = so_src_local[m], so_norm[m], so_tl[m]
        grp = cb * nq + cq
        pos = _group_cumcount(grp)

        idx16 = np.zeros((16, IW), dtype=np.int16)
        tn = np.zeros((P, 2 * K_total), dtype=np.float32)
        for b in range(n_blocks):
            o = tn_off[b]
            tn[:, o : o + k_b[b]] = -1.0

        idx16[pos % 16, ix_off[cb, cq] + pos // 16] = csl.astype(np.int16)

        slot_in_block = gslot_off[cb, cq] + pos // P
        prow = pos % P
        tn[prow, tn_off[cb] + slot_in_block] = ctl.astype(np.float32)
        tn[prow, tn_off[cb] + k_b[cb] + slot_in_block] = cn

        per_core.append({"idx16": np.tile(idx16, (8, 1)), "tn": tn})

    return {
        "slots": slots,
        "k_b": k_b,
        "K_total": K_total,
        "IW": IW,
        "tn_off": tn_off,
        "ix_off": ix_off,
        "gslot_off": gslot_off,
        "n_blocks": n_blocks,
        "nq": nq,
        "per_core": per_core,
    }


def _build_program(meta, n_nodes: int, chunk: int, fin, fh, fout):
    N = n_nodes
    S = N // NC
    n_blocks = meta["n_blocks"]
    nq = meta["nq"]
    slots = meta["slots"]
    k_b = meta["k_b"]
    K_total = meta["K_total"]
    IW = meta["IW"]
    tn_off = meta["tn_off"]
    ix_off = meta["ix_off"]
    gslot_off = meta["gslot_off"]
    fo_pad = 64

    nc = bacc.Bacc()

    xT = nc.dram_tensor("xT", [fin, S], F32, kind="ExternalInput")
    W1 = nc.dram_tensor("W1", [fin, fh], F32, kind="ExternalInput")
    W2 = nc.dram_tensor("W2", [fh, fh], F32, kind="ExternalInput")
    W3 = nc.dram_tensor("W3", [fh, fout], F32, kind="ExternalInput")
    LW = nc.dram_tensor("LW", [2 * fh + fout, fout], F32, kind="ExternalInput")
    idx16 = nc.dram_tensor("idx16", [P, IW], I16, kind="ExternalInput")
    tn = nc.dram_tensor("tn", [P, 2 * K_total], F32, kind="ExternalInput")
    iota_in = nc.dram_tensor("iota", [P, P], F32, kind="ExternalInput")
    ident_in = nc.dram_tensor("ident", [P, P], F32, kind="ExternalInput")
    b1bc_in = nc.dram_tensor("b1bc", [P, fh], F32, kind="ExternalInput")
    b2bc_in = nc.dram_tensor("b2bc", [P, fh], F32, kind="ExternalInput")
    b3bc_in = nc.dram_tensor("b3bc", [P, fout], F32, kind="ExternalInput")
    lbbc_in = nc.dram_tensor("lbbc", [P, fout], F32, kind="ExternalInput")
    out_sh = nc.dram_tensor("out_sh", [S, fout], F32, kind="ExternalOutput")

    t1_sh = nc.dram_tensor("t1_sh", [S, fh], F32)
    t2_sh = nc.dram_tensor("t2_sh", [S, fh], F32)
    t3_sh = nc.dram_tensor("t3_sh", [S, fo_pad], F32)
    t1_full = nc.dram_tensor("t1_full", [N, fh], F32, addr_space="Shared")
    t2_full = nc.dram_tensor("t2_full", [N, fh], F32, addr_space="Shared")
    t3_full = nc.dram_tensor("t3_full", [N, fo_pad], F32, addr_space="Shared")
    h1T_sh = nc.dram_tensor("h1T_sh", [fh, S], F32)
    h2T_sh = nc.dram_tensor("h2T_sh", [fh, S], F32)

    rg = [list(range(NC))]

    def used_rows(b):
        return min(P, S - b * P)

    with tile.TileContext(nc) as tc:
        with (
            tc.tile_pool(name="const", bufs=1) as cpool,
            tc.tile_pool(name="sb", bufs=3) as pool,
            tc.tile_pool(name="gath", bufs=3) as gpool,
            tc.tile_pool(name="ps", bufs=2, space="PSUM") as psp,
        ):
            iota_t = cpool.tile([P, P], F32)
            nc.sync.dma_start(out=iota_t[:], in_=iota_in[:, :])
            ident_t = cpool.tile([P, P], F32)
            nc.sync.dma_start(out=ident_t[:], in_=ident_in[:, :])
            w1_t = cpool.tile([P, 2, fh], F32)
            nc.sync.dma_start(out=w1_t[:], in_=W1[:, :].rearrange("(c k) f -> k c f", k=P))
            w2_t = cpool.tile([P, 2, fh], F32)
            nc.sync.dma_start(out=w2_t[:], in_=W2[:, :].rearrange("(c k) f -> k c f", k=P))
            w3_t = cpool.tile([P, 2, fout], F32)
            nc.sync.dma_start(out=w3_t[:], in_=W3[:, :].rearrange("(c k) f -> k c f", k=P))
            lw12_t = cpool.tile([P, 4, fout], F32)
            nc.sync.dma_start(
                out=lw12_t[:], in_=LW[: 4 * P, :].rearrange("(c k) f -> k c f", k=P)
            )
            lw3_t = cpool.tile([fout, fout], F32)
            nc.sync.dma_start(out=lw3_t[:], in_=LW[4 * P :, :])
            b1bc = cpool.tile([P, fh], F32)
            nc.sync.dma_start(out=b1bc[:], in_=b1bc_in[:, :])
            b2bc = cpool.tile([P, fh], F32)
            nc.sync.dma_start(out=b2bc[:], in_=b2bc_in[:, :])
            b3bc = cpool.tile([P, fout], F32)
            nc.sync.dma_start(out=b3bc[:], in_=b3bc_in[:, :])
            lbbc = cpool.tile([P, fout], F32)
            nc.sync.dma_start(out=lbbc[:], in_=lbbc_in[:, :])

            for b in range(n_blocks):
                u = used_rows(b)
                ps1 = psp.tile([P, fh], F32, tag="ps2")
                for cc in range(2):
                    xt = pool.tile([P, P], F32, tag="xt")
                    nc.sync.dma_start(
                        out=xt[:, :u], in_=xT[cc * P : (cc + 1) * P, b * P : b * P + u]
                    )
                    nc.tensor.matmul(
                        ps1[:u, :],
                        lhsT=xt[:, :u],
                        rhs=w1_t[:, cc, :],
                        start=(cc == 0),
                        stop=(cc == 1),
                    )
                ev = pool.tile([P, fh], F32, tag="ev")
                nc.scalar.activation(ev[:u, :], ps1[:u, :], AF.Copy)
                nc.sync.dma_start(out=t1_sh[b * P : b * P + u, :], in_=ev[:u, :])

            nc.gpsimd.collective_compute(
                "AllGather", AL.bypass, ins=[t1_sh[:, :]], outs=[t1_full[:, :]],
                replica_groups=rg,
            )

            def layer(li, table, felem, wnext_t, fnext, bias_bc, tnext_sh, hT_sh_):
                for b in range(n_blocks):
                    u = used_rows(b)
                    kb = int(k_b[b])
                    tno = int(tn_off[b])
                    tnt = pool.tile([P, 2 * kb], F32, tag="tnt")
                    nc.sync.dma_start(out=tnt[:], in_=tn[:, tno : tno + 2 * kb])
                    dsts = []
                    for qq in range(nq):
                        sl = int(slots[b, qq])
                        if sl == 0:
                            dsts.append(None)
                            continue
                        nidx = sl * P
                        ixt = pool.tile([P, sl * 8], I16, tag=f"ix{qq}")
                        io = int(ix_off[b, qq])
                        nc.sync.dma_start(out=ixt[:], in_=idx16[:, io : io + sl * 8])
                        dst = gpool.tile([P, sl, felem], F32, tag=f"dst{qq}")
                        base = qq * chunk
                        rows = min(chunk, N - base)
                        nc.gpsimd.dma_gather(
                            dst[:],
                            table[base : base + rows, :],
                            ixt[:],
                            nidx,
                            nidx,
                            felem,
                            single_packet=False,
                        )
                        dsts.append(dst)
                    fagg = fh if li < 3 else fout
                    psa = psp.tile([P, fagg], F32, tag="psa")
                    first = True
                    for qq in range(nq):
                        sl = int(slots[b, qq])
                        if sl == 0:
                            continue
                        go = int(gslot_off[b, qq])
                        for j in range(sl):
                            s = go + j
                            st = pool.tile([P, P], F32, tag="st")
                            nc.vector.tensor_scalar(
                                out=st[:],
                                in0=iota_t[:],
                                scalar1=tnt[:, s : s + 1],
                                scalar2=tnt[:, kb + s : kb + s + 1],
                                op0=AL.is_equal,
                                op1=AL.mult,
                            )
                            nc.tensor.matmul(
                                psa[:],
                                lhsT=st[:],
                                rhs=dsts[qq][:, j, :fagg],
                                start=first,
                                stop=False,
                            )
                            first = False
                    nc.tensor.matmul(
                        psa[:], lhsT=ident_t[:], rhs=bias_bc[:, :fagg],
                        start=False, stop=True,
                    )
                    h_sb = pool.tile([P, fagg], F32, tag="h_sb")
                    nc.scalar.activation(h_sb[:], psa[:], AF.Relu)

                    if li < 3:
                        ps2 = psp.tile([P, fnext], F32, tag="ps2")
                        for cc in range(2):
                            pst = psp.tile([P, P], F32, tag=f"pst{cc}")
                            nc.tensor.transpose(
                                pst[:], h_sb[:, cc * P : (cc + 1) * P], ident_t[:]
                            )
                            hT = pool.tile([P, P], F32, tag=f"hT{cc}")
                            nc.vector.tensor_copy(hT[:], pst[:])
                            nc.sync.dma_start(
                                out=hT_sh_[cc * P : (cc + 1) * P, b * P : b * P + u],
                                in_=hT[:, :u],
                            )
                            nc.tensor.matmul(
                                ps2[:u, :],
                                lhsT=hT[:, :u],
                                rhs=wnext_t[:, cc, :fnext],
                                start=(cc == 0),
                                stop=(cc == 1),
                            )
                        ev2 = pool.tile([P, fnext], F32, tag="ev")
                        nc.scalar.activation(ev2[:u, :fnext], ps2[:u, :], AF.Copy)
                        nc.sync.dma_start(
                            out=tnext_sh[b * P : b * P + u, :fnext], in_=ev2[:u, :fnext]
                        )
                    else:
                        ps3t = psp.tile([fout, P], F32, tag="pst0")
                        nc.tensor.transpose(ps3t[:], h_sb[:, :fout], ident_t[:])
                        h3T = pool.tile([fout, P], F32, tag="hT0")
                        nc.vector.tensor_copy(h3T[:], ps3t[:])
                        pso = psp.tile([P, fout], F32, tag="ps2")
                        for cc in range(2):
                            r1 = pool.tile([P, P], F32, tag=f"rl{cc}")
                            nc.sync.dma_start(
                                out=r1[:, :u],
                                in_=h1T_sh[cc * P : (cc + 1) * P, b * P : b * P + u],
                            )
                            nc.tensor.matmul(
                                pso[:u, :], lhsT=r1[:, :u], rhs=lw12_t[:, cc, :],
                                start=(cc == 0), stop=False,
                            )
                        for cc in range(2):
                            r2 = pool.tile([P, P], F32, tag=f"rl{2 + cc}")
                            nc.sync.dma_start(
                                out=r2[:, :u],
                                in_=h2T_sh[cc * P : (cc + 1) * P, b * P : b * P + u],
                            )
                            nc.tensor.matmul(
                                pso[:u, :], lhsT=r2[:, :u], rhs=lw12_t[:, 2 + cc, :],
                                start=False, stop=False,
                            )
                        nc.tensor.matmul(
                            pso[:u, :], lhsT=h3T[:, :u], rhs=lw3_t[:, :],
                            start=False, stop=False,
                        )
                        nc.tensor.matmul(
                            pso[:u, :], lhsT=ident_t[:, :u], rhs=lbbc[:, :],
                            start=False, stop=True,
                        )
                        m_t = pool.tile([P, 1], F32, tag="m_t")
                        nc.vector.tensor_reduce(
                            m_t[:u, :], pso[:u, :], mybir.AxisListType.X, AL.max
                        )
                        nm_t = pool.tile([P, 1], F32, tag="nm_t")
                        nc.vector.tensor_scalar(
                            out=nm_t[:u, :], in0=m_t[:u, :], scalar1=-1.0,
                            scalar2=None, op0=AL.mult,
                        )
                        e_t = pool.tile([P, fout], F32, tag="e_t")
                        ssum = pool.tile([P, 1], F32, tag="ssum")
                        nc.scalar.activation(
                            e_t[:u, :], pso[:u, :], AF.Exp,
                            bias=nm_t[:u, :1], accum_out=ssum[:u, :1],
                        )
                        ls_t = pool.tile([P, 1], F32, tag="ls_t")
                        nc.scalar.activation(ls_t[:u, :], ssum[:u, :], AF.Ln)
                        mls = pool.tile([P, 1], F32, tag="mls")
                        nc.vector.tensor_tensor(
                            out=mls[:u, :], in0=m_t[:u, :], in1=ls_t[:u, :], op=AL.add
                        )
                        z_t = pool.tile([P, fout], F32, tag="z_t")
                        nc.vector.tensor_scalar(
                            out=z_t[:u, :], in0=pso[:u, :], scalar1=mls[:u, :1],
                            scalar2=None, op0=AL.subtract,
                        )
                        nc.sync.dma_start(
                            out=out_sh[b * P : b * P + u, :], in_=z_t[:u, :]
                        )

            layer(1, t1_full, fh, w2_t, fh, b1bc, t2_sh, h1T_sh)
            nc.gpsimd.collective_compute(
                "AllGather", AL.bypass, ins=[t2_sh[:, :]], outs=[t2_full[:, :]],
                replica_groups=rg,
            )
            layer(2, t2_full, fh, w3_t, fout, b2bc, t3_sh, h2T_sh)
            nc.gpsimd.collective_compute(
                "AllGather", AL.bypass, ins=[t3_sh[:, :]], outs=[t3_full[:, :]],
                replica_groups=rg,
            )
            layer(3, t3_full, fo_pad, None, None, b3bc, None, None)

    nc.finalize()
    return nc


def kernel(x, edge_index, W1, b1, W2, b2, W3, b3, lin_w, lin_b):
    global LAST_EXEC_NS
    x = np.asarray(x)
    N = x.shape[0]
    S = N // NC
    fin, fh, fout = W1.shape[0], W2.shape[0], W3.shape[1]

    meta = _preprocess(np.asarray(edge_index, dtype=np.int64), N, CHUNK)
    nc = _build_program(meta, N, CHUNK, fin, fh, fout)

    iota = np.tile(np.arange(P, dtype=np.float32), (P, 1))
    ident = np.eye(P, dtype=np.float32)
    b1bc = np.tile(np.asarray(b1, np.float32), (P, 1))
    b2bc = np.tile(np.asarray(b2, np.float32), (P, 1))
    b3bc = np.tile(np.asarray(b3, np.float32), (P, 1))
    lbbc = np.tile(np.asarray(lin_b, np.float32), (P, 1))

    in_maps = []
    for c in range(NC):
        xs = np.asarray(x[c * S : (c + 1) * S], np.float32)
        in_maps.append(
            {
                "xT": np.ascontiguousarray(xs.T),
                "W1": np.asarray(W1, np.float32),
                "W2": np.asarray(W2, np.float32),
                "W3": np.asarray(W3, np.float32),
                "LW": np.asarray(lin_w, np.float32),
                "idx16": meta["per_core"][c]["idx16"],
                "tn": meta["per_core"][c]["tn"],
                "iota": iota,
                "ident": ident,
                "b1bc": b1bc,
                "b2bc": b2bc,
                "b3bc": b3bc,
                "lbbc": lbbc,
            }
        )
    trace = bool(os.environ.get("GCN_TRACE"))
    res = run_bass_kernel_spmd(nc, in_maps, list(range(NC)), trace=trace)
    LAST_EXEC_NS = res.exec_time_ns
    out = np.concatenate([res.results[c]["out_sh"] for c in range(NC)], axis=0)
    return out.astype(np.float32)
